# revision 13
# baseline (speedup 1.0000x reference)
"""AttentionLSEncoder (landmark + sliding-window attention) on 8 TRN2
NeuronCores, data-parallel over the batch (b=8 -> 1 element per core).

kernel(**inputs) takes the FULL unsharded inputs and returns the FULL
[8, 2000, 512] float32 output.
"""
import numpy as np
import ml_dtypes

import concourse.bass as bass
import concourse.tile as tile
from concourse import mybir
from concourse.bass_utils import run_bass_kernel_spmd

# ----------------------------------------------------------------------------
# Workaround: this container's walrus build accepts only ONE sync-wait per
# instruction. Split multi-wait instructions into single-wait NoOp chains.
# ----------------------------------------------------------------------------
from concourse.vector_clock import ScopedClock

_orig_add = tile.TileContext._add_instruction


def _split_waits_engine(self, inst):
    si = getattr(inst, "sync_info", None)
    if si is None or not si.on_wait or len(si.on_wait) <= 1:
        return
    eng = inst.engine
    if eng is None or eng == mybir.EngineType.Unassigned:
        return
    waits = list(si.on_wait)
    for i, w in enumerate(waits[:-1]):
        nop = mybir.InstNoOp(
            name=f"{inst.name}-wsplit{i}",
            sync_info=mybir.SyncInfo(on_wait=[w], on_update=[]),
            bass_nofuse=True,
            engine=eng,
        )
        _orig_add(self, nop)
    inst.sync_info = mybir.SyncInfo(
        on_wait=[waits[-1]], on_update=list(si.on_update or [])
    )


def _add_instruction_split(self, inst):
    _split_waits_engine(self, inst)
    _orig_add(self, inst)


def _drain_and_barrier_split(self, tick_clock, wait_clock):
    nc = self.nc
    drain_inst = nc.sync.drain()
    wait_clock.add_sem_waits(
        drain_inst.ins, ScopedClock({None: tick_clock.global_clock})
    )
    si = drain_inst.ins.sync_info
    waits = list(si.on_wait) if si and si.on_wait else []
    if len(waits) > 1:
        drain_inst.ins.sync_info = mybir.SyncInfo(
            on_wait=waits[:1], on_update=list(si.on_update or [])
        )
        for w in waits[1:]:
            extra = nc.sync.drain()
            extra.ins.sync_info = mybir.SyncInfo(on_wait=[w], on_update=[])

    nc.all_engine_barrier()
    assert self.sems is not None
    popped = nc._tile_sem_poison_stack.pop()
    assert popped is self._sem_poison
    nc.clear_and_free_semaphores(list(self.sems.allocated().values()))
    nc.all_engine_barrier()


tile.TileContext._add_instruction = _add_instruction_split
tile.TileContext._drain_and_barrier = _drain_and_barrier_split

# ----------------------------------------------------------------------------
# Problem constants (hardcoded per the module definition)
# ----------------------------------------------------------------------------
B = 8
S = 2000
DM = 512
H = 4
DK = 512
HD = H * DK          # 2048
NL = 32
HL = H * NL          # 128
WS = 10
EXT = 5

NB = 20              # attention blocks
BT = 100             # tokens per block
BW = 110             # band keys per block
SW = NL + BW         # score width = 142
ND = DM // 128       # 4 contraction chunks
NF = HD // 128       # 16 feature chunks
NG = 4               # token groups of 500
GT = S // NG         # 500
SCALE = 1.0 / float(np.sqrt(DK))

# token tiles of 128 (last is 80)
TT = [(i * 128, min(128, S - i * 128)) for i in range((S + 127) // 128)]
NT = len(TT)  # 16

F32 = mybir.dt.float32
F32R = mybir.dt.float32r
BF16 = mybir.dt.bfloat16
BF = ml_dtypes.bfloat16


def _band_start(i):
    return min(max(BT * i - EXT, 0), S - BW)


def _host_consts():
    eye = np.eye(128, dtype=BF)
    g = np.zeros((WS, BT), dtype=BF)
    for w in range(BT):
        g[w // WS, w] = 1.0
    masks = np.full((WS, NB, SW), -30.0, dtype=np.float32)
    masks[:, :, :NL] = 0.0
    for i in range(NB):
        b0 = _band_start(i)
        for gw in range(WS):
            gg = WS * i + gw
            lo, hi = WS * gg - EXT, WS * gg + EXT + WS
            for j in range(BW):
                k = b0 + j
                if lo <= k < hi:
                    masks[gw, i, NL + j] = 0.0
    return eye, g, masks.astype(BF)


def build_nc(debug=False):
    nc = bass.Bass("TRN2", target_bir_lowering=False, debug=False)

    xt = nc.dram_tensor("xt", [DM, S], F32R, kind="ExternalInput")
    wq = nc.dram_tensor("wq", [DM, HD], F32R, kind="ExternalInput")
    wk = nc.dram_tensor("wk", [DM, HD], F32R, kind="ExternalInput")
    wv = nc.dram_tensor("wv", [DM, HD], F32R, kind="ExternalInput")
    wd = nc.dram_tensor("wd", [DM, HL], F32R, kind="ExternalInput")
    wo = nc.dram_tensor("wo", [HD, DM], BF16, kind="ExternalInput")
    eyeb = nc.dram_tensor("eyeb", [128, 128], BF16, kind="ExternalInput")
    gmat = nc.dram_tensor("gmat", [WS, BT], BF16, kind="ExternalInput")
    mks = nc.dram_tensor("mks", [WS, NB, SW], BF16, kind="ExternalInput")
    out = nc.dram_tensor("out", [S, DM], F32, kind="ExternalOutput")

    k_bf = nc.dram_tensor("k_bf", [S, HD], BF16, kind="ExternalOutput" if debug else "Internal")
    v_bf = nc.dram_tensor("v_bf", [S, HD], BF16, kind="ExternalOutput" if debug else "Internal")
    if debug:
        dbg_qt = nc.dram_tensor("dbg_qt", [128, NF, S], BF16, kind="ExternalOutput")
        dbg_hs = nc.dram_tensor("dbg_hs", [128, S], BF16, kind="ExternalOutput")
        dbg_rlm = nc.dram_tensor("dbg_rlm", [128, 1], F32, kind="ExternalOutput")
        dbg_kc = nc.dram_tensor("dbg_kc", [32, HD], BF16, kind="ExternalOutput")
        dbg_vc = nc.dram_tensor("dbg_vc", [32, HD], BF16, kind="ExternalOutput")
        dbg_kct = nc.dram_tensor("dbg_kct", [128, NF, NL], BF16, kind="ExternalOutput")
        dbg_kt = nc.dram_tensor("dbg_kt", [128, NF, S], BF16, kind="ExternalOutput")
        dbg_sc = nc.dram_tensor("dbg_sc", [BT, SW], F32, kind="ExternalOutput")
        dbg_e = nc.dram_tensor("dbg_e", [BT, SW], BF16, kind="ExternalOutput")
        dbg_et = nc.dram_tensor("dbg_et", [BW, 2, BT], BF16, kind="ExternalOutput")
        dbg_ct = nc.dram_tensor("dbg_ct", [128, NF, BT], BF16, kind="ExternalOutput")

    xt_r = xt.ap().rearrange("(c p) t -> p c t", p=128)

    with tile.TileContext(nc) as tc:
        with (
            tc.tile_pool(name="resid", bufs=1) as rp,
            tc.tile_pool(name="rpsum", bufs=1, space="PSUM") as _unused_rps,
        ):
            eye_sb = rp.tile([128, 128], BF16)
            nc.sync.dma_start(eye_sb[:], eyeb.ap())
            g_sb = rp.tile([WS, BT], BF16)
            nc.sync.dma_start(g_sb[:], gmat.ap())
            eps = rp.tile([128, 1], F32)
            nc.vector.memset(eps[:], 1e-5)

            qt = rp.tile([128, NF, S], BF16)
            kt = rp.tile([128, NF, S], BF16)
            hs_exp = rp.tile([128, S], BF16)
            recip_lm = rp.tile([128, 1], F32)
            kct = rp.tile([128, NF, NL], BF16)
            vc = rp.tile([32, HD], BF16)
            kc_tok = rp.tile([32, HD], BF16)

            # ================= S1a: K/V projections + LN, hs logits ======
            with (
                tc.tile_pool(name="s1a", bufs=1) as p1,
                tc.tile_pool(name="s1aw", bufs=2) as p1w,
                tc.tile_pool(name="ps1", bufs=2, space="PSUM") as ps1,
            ):
                wd_sb = p1.tile([128, ND, HL], F32R)
                nc.sync.dma_start(
                    wd_sb[:], wd.ap().rearrange("(c p) f -> p c f", p=128)
                )
                den4 = p1.tile([128, NG], F32)

                for tens_i, (wdram, odram) in enumerate(((wk, k_bf), (wv, v_bf))):
                    w_sb = p1.tile([128, ND, HD], F32R, tag="wtens")
                    nc.sync.dma_start(
                        w_sb[:], wdram.ap().rearrange("(c p) f -> p c f", p=128)
                    )
                    for ti, (t0, tw) in enumerate(TT):
                        xt_t = p1w.tile([128, ND, 128], F32R, tag="xt_t")
                        nc.sync.dma_start(xt_t[:, :, :tw], xt_r[:, :, t0 : t0 + tw])
                        pra = ps1.tile([128, 2, 512], F32, tag="projA")
                        prb = ps1.tile([128, 2, 512], F32, tag="projB")
                        halves = (pra, prb)
                        for g4 in range(ND):
                            pr = halves[g4 // 2]
                            for c in range(ND):
                                nc.tensor.matmul(
                                    pr[:tw, g4 % 2, :],
                                    xt_t[:, c, :tw],
                                    w_sb[:, c, 512 * g4 : 512 * (g4 + 1)],
                                    start=(c == 0),
                                    stop=(c == ND - 1),
                                )
                        st = p1w.tile([128, ND, 6], F32, tag="st")
                        for g4 in range(ND):
                            nc.vector.bn_stats(
                                st[:tw, g4, :], halves[g4 // 2][:tw, g4 % 2, :]
                            )
                        mv = p1w.tile([128, 2], F32, tag="mv")
                        nc.vector.bn_aggr(mv[:tw], st[:tw])
                        sd = p1w.tile([128, 1], F32, tag="sd")
                        nc.scalar.activation(
                            sd[:tw], mv[:tw, 1:2],
                            mybir.ActivationFunctionType.Sqrt, bias=eps[:tw],
                        )
                        rs = p1w.tile([128, 1], F32, tag="rs")
                        nc.vector.reciprocal(rs[:tw], sd[:tw])
                        stg = p1w.tile([128, HD], BF16, tag="stg")
                        for half in range(2):
                            nc.vector.tensor_scalar(
                                stg[:tw, 1024 * half : 1024 * (half + 1)],
                                halves[half][:tw], mv[:tw, 0:1], rs[:tw],
                                mybir.AluOpType.subtract, mybir.AluOpType.mult,
                            )
                        nc.sync.dma_start(odram[t0 : t0 + tw, :], stg[:tw])

                # landmark logits -> exp (feature-major [HL, S])
                for g4 in range(NG):
                    xt_g = p1w.tile([128, ND, GT], F32R, tag="xt_g")
                    nc.sync.dma_start(xt_g[:], xt_r[:, :, GT * g4 : GT * (g4 + 1)])
                    hp = ps1.tile([128, GT], F32, tag="projA")
                    for c in range(ND):
                        nc.tensor.matmul(
                            hp[:], wd_sb[:, c, :], xt_g[:, c, :],
                            start=(c == 0), stop=(c == ND - 1),
                        )
                    nc.scalar.activation(
                        hs_exp[:, GT * g4 : GT * (g4 + 1)], hp[:],
                        mybir.ActivationFunctionType.Exp,
                        accum_out=den4[:, g4 : g4 + 1],
                    )
                dsum = p1.tile([128, 1], F32)
                nc.vector.reduce_sum(dsum[:], den4[:], axis=mybir.AxisListType.X)
                nc.vector.reciprocal(recip_lm[:], dsum[:])

            # ================= S1b: Q^T projection (feature-major) =======
            with (
                tc.tile_pool(name="s1b", bufs=1) as p2,
                tc.tile_pool(name="s1bw", bufs=2) as p2w,
                tc.tile_pool(name="ps2", bufs=4, space="PSUM") as ps2,
            ):
                wq_sb = p2.tile([128, ND, HD], F32R)
                nc.sync.dma_start(
                    wq_sb[:], wq.ap().rearrange("(c p) f -> p c f", p=128)
                )
                for g4 in range(NG):
                    xt_g = p2w.tile([128, ND, GT], F32R, tag="xt_g")
                    nc.sync.dma_start(xt_g[:], xt_r[:, :, GT * g4 : GT * (g4 + 1)])
                    for j in range(NF):
                        qp = ps2.tile([128, GT], F32, tag="q")
                        for c in range(ND):
                            nc.tensor.matmul(
                                qp[:], wq_sb[:, c, 128 * j : 128 * (j + 1)],
                                xt_g[:, c, :],
                                start=(c == 0), stop=(c == ND - 1),
                            )
                        nc.scalar.activation(
                            qt[:, j, GT * g4 : GT * (g4 + 1)], qp[:],
                            mybir.ActivationFunctionType.Copy, scale=SCALE,
                        )

            # ================= S2: K^T, hs transpose, K_c/V_c ============
            with (
                tc.tile_pool(name="s2", bufs=1) as p3,
                tc.tile_pool(name="s2w", bufs=3) as p3w,
                tc.tile_pool(name="ps3", bufs=1, space="PSUM") as ps3,
                tc.tile_pool(name="ps3t", bufs=2, space="PSUM") as ps3t,
            ):
                # hs_exp [128, S] -> hs_tok [s, 128] per 128-token chunk
                hs_tok = p3.tile([128, NT, HL], BF16)
                for ti, (t0, tw) in enumerate(TT):
                    tp = ps3t.tile([128, 128], BF16, tag="tr")
                    nc.tensor.transpose(
                        tp[:tw, :], hs_exp[:, t0 : t0 + tw], eye_sb[:]
                    )
                    nc.scalar.copy(hs_tok[:tw, ti, :], tp[:tw, :])

                # K_c then V_c
                for tens_i, (src, dst) in enumerate(
                    ((k_bf, kc_tok), (v_bf, vc))
                ):
                    cv = ps3.tile([32, HD], F32, tag="cvec")
                    for ti, (t0, tw) in enumerate(TT):
                        ch = p3w.tile([128, HD], BF16, tag="chunk")
                        nc.sync.dma_start(ch[:tw], src[t0 : t0 + tw, :])
                        for h in range(H):
                            nc.tensor.matmul(
                                cv[:, 512 * h : 512 * (h + 1)],
                                hs_tok[:tw, ti, 32 * h : 32 * (h + 1)],
                                ch[:tw, 512 * h : 512 * (h + 1)],
                                start=(ti == 0), stop=(ti == NT - 1),
                            )
                        if tens_i == 0:
                            # K^T via on-chip PE transposes of each chunk
                            for j in range(NF):
                                tp = ps3t.tile([128, 128], BF16, tag="tr")
                                nc.tensor.transpose(
                                    tp[:, :tw],
                                    ch[:tw, 128 * j : 128 * (j + 1)],
                                    eye_sb[:tw, :tw],
                                )
                                nc.scalar.copy(
                                    kt[:, j, t0 : t0 + tw], tp[:, :tw]
                                )
                    # scale by landmark softmax denominators, then LN
                    for h in range(H):
                        nc.vector.tensor_scalar(
                            cv[:, 512 * h : 512 * (h + 1)],
                            cv[:, 512 * h : 512 * (h + 1)],
                            recip_lm[32 * h : 32 * (h + 1), 0:1], None,
                            mybir.AluOpType.mult,
                        )
                    st2 = p3w.tile([32, ND, 6], F32, tag="st2")
                    for g4 in range(ND):
                        nc.vector.bn_stats(st2[:, g4, :], cv[:, 512 * g4 : 512 * (g4 + 1)])
                    mv2 = p3w.tile([32, 2], F32, tag="mv2")
                    nc.vector.bn_aggr(mv2[:], st2[:])
                    sd2 = p3w.tile([32, 1], F32, tag="sd2")
                    nc.scalar.activation(
                        sd2[:], mv2[:, 1:2],
                        mybir.ActivationFunctionType.Sqrt, bias=eps[:32],
                    )
                    rs2 = p3w.tile([32, 1], F32, tag="rs2")
                    nc.vector.reciprocal(rs2[:], sd2[:])
                    nc.vector.tensor_scalar(
                        dst[:], cv[:], mv2[:, 0:1], rs2[:],
                        mybir.AluOpType.subtract, mybir.AluOpType.mult,
                    )

                # K_c^T [f, l] for the score matmuls
                for j in range(NF):
                    tp = ps3t.tile([128, NL], BF16, tag="trc")
                    nc.tensor.transpose(
                        tp[:], kc_tok[:, 128 * j : 128 * (j + 1)], eye_sb[:32, :32]
                    )
                    nc.scalar.copy(kct[:, j, :], tp[:])
                if debug:
                    nc.sync.dma_start(dbg_qt.ap(), qt[:])
                    nc.sync.dma_start(dbg_hs.ap(), hs_exp[:])
                    nc.sync.dma_start(dbg_rlm.ap(), recip_lm[:])
                    nc.sync.dma_start(dbg_kc.ap(), kc_tok[:])
                    nc.sync.dma_start(dbg_vc.ap(), vc[:])
                    nc.sync.dma_start(dbg_kct.ap(), kct[:])
                    nc.sync.dma_start(dbg_kt.ap(), kt[:])

            # ================= S4: blocked attention + Wo ================
            with (
                tc.tile_pool(name="s4", bufs=1) as p4,
                tc.tile_pool(name="s4w", bufs=2) as p4w,
                tc.tile_pool(name="ps4", bufs=2, space="PSUM") as ps4,
                tc.tile_pool(name="ps4b", bufs=1, space="PSUM") as ps4b,
                tc.tile_pool(name="ps4o", bufs=1, space="PSUM") as ps4o,
            ):
                wo_sb = p4.tile([128, NF, DM], BF16)
                nc.sync.dma_start(
                    wo_sb[:], wo.ap().rearrange("(c p) f -> p c f", p=128)
                )
                m_sb = p4.tile([WS, NB, SW], BF16)
                nc.sync.dma_start(m_sb[:], mks.ap())

                for i in range(NB):
                    t0 = BT * i
                    b0 = _band_start(i)
                    vband = p4w.tile([BW, HD], BF16, tag="vband")
                    nc.sync.dma_start(vband[:], v_bf[b0 : b0 + BW, :])
                    # 128-col chunk stride keeps each head's 4 chunks in
                    # exactly one PSUM bank (start=True clears a whole bank
                    # on this HW, so one start per bank, first toucher).
                    ct = ps4o.tile([128, NF, 128], F32, tag="ct")
                    for h in range(H):
                        sc = ps4.tile([BT, SW], F32, tag="sc")
                        # mask first: start=True writes every score column,
                        # then all QK matmuls accumulate.
                        nc.tensor.matmul(
                            sc[:], g_sb[:], m_sb[:, i, :],
                            start=True, stop=False, skip_group_check=True,
                        )
                        for c4 in range(ND):
                            c = 4 * h + c4
                            nc.tensor.matmul(
                                sc[:, :NL], qt[:, c, t0 : t0 + BT], kct[:, c, :],
                                start=False, stop=False,
                                skip_group_check=True,
                            )
                            nc.tensor.matmul(
                                sc[:, NL:], qt[:, c, t0 : t0 + BT],
                                kt[:, c, b0 : b0 + BW],
                                start=False, stop=(c4 == ND - 1),
                                skip_group_check=True,
                            )
                        e_sb = p4w.tile([BT, SW], BF16, tag="e")
                        den = p4w.tile([BT, 1], F32, tag="den")
                        nc.scalar.activation(
                            e_sb[:], sc[:], mybir.ActivationFunctionType.Exp,
                            accum_out=den[:],
                        )
                        rec = p4w.tile([BT, 1], F32, tag="rec")
                        nc.vector.reciprocal(rec[:], den[:])
                        d_sb = p4w.tile([BT, BT], BF16, tag="d")
                        nc.vector.tensor_scalar(
                            d_sb[:], eye_sb[:BT, :BT], rec[:], None,
                            mybir.AluOpType.mult,
                        )
                        etp = ps4b.tile([BW, 2, BT], F32, tag="et")
                        # band transpose first: its start=True clears the
                        # bank; the landmark transpose then overwrites its
                        # own (cleared) region with start=False.
                        nc.tensor.matmul(
                            etp[:, 1, :], e_sb[:, NL:], d_sb[:],
                            start=True, stop=False, skip_group_check=True,
                        )
                        nc.tensor.matmul(
                            etp[:NL, 0, :], e_sb[:, :NL], d_sb[:],
                            start=False, stop=True, skip_group_check=True,
                        )
                        et_sb = p4w.tile([BW, 2, BT], BF16, tag="ets")
                        nc.scalar.copy(et_sb[:], etp[:])
                        if debug and i == 0 and h == 0:
                            scd = p4w.tile([BT, SW], F32, tag="scdump")
                            nc.scalar.copy(scd[:], sc[:])
                            nc.sync.dma_start(dbg_sc.ap(), scd[:])
                            nc.sync.dma_start(dbg_e.ap(), e_sb[:])
                            nc.sync.dma_start(dbg_et.ap(), et_sb[:])
                        for c4 in range(ND):
                            c = 4 * h + c4
                            d0 = 512 * h + 128 * c4
                            nc.tensor.matmul(
                                ct[:, c, :BT], vc[:, d0 : d0 + 128],
                                et_sb[:NL, 0, :],
                                start=(c4 == 0), stop=False,
                                skip_group_check=True,
                            )
                            nc.tensor.matmul(
                                ct[:, c, :BT], vband[:, d0 : d0 + 128],
                                et_sb[:, 1, :],
                                start=False, stop=(c4 == ND - 1),
                                skip_group_check=True,
                            )
                    ct_sb = p4w.tile([128, NF, BT], BF16, tag="cts")
                    nc.scalar.copy(ct_sb[:], ct[:, :, :BT])
                    if debug and i == 0:
                        nc.sync.dma_start(dbg_ct.ap(), ct_sb[:])
                    op = ps4o.tile([BT, DM], F32, tag="wo")
                    for c in range(NF):
                        nc.tensor.matmul(
                            op[:], ct_sb[:, c, :], wo_sb[:, c, :],
                            start=(c == 0), stop=(c == NF - 1),
                        )
                    o_sb = p4w.tile([BT, DM], F32, tag="osb")
                    nc.scalar.copy(o_sb[:], op[:])
                    nc.sync.dma_start(out[t0 : t0 + BT, :], o_sb[:])

    return nc


_NC_CACHE = {}


def _get_nc():
    if "nc" not in _NC_CACHE:
        _NC_CACHE["nc"] = build_nc()
    return _NC_CACHE["nc"]


def kernel(**inputs):
    X = np.asarray(inputs["X"], dtype=np.float32)
    Wq = np.asarray(inputs["Wq"], dtype=np.float32)
    Wk = np.asarray(inputs["Wk"], dtype=np.float32)
    Wv = np.asarray(inputs["Wv"], dtype=np.float32)
    Wd = np.asarray(inputs["Wd"], dtype=np.float32)
    Wo = np.asarray(inputs["Wo"], dtype=np.float32)

    eye, g, masks = _host_consts()
    shared = {
        "wq": Wq, "wk": Wk, "wv": Wv, "wd": Wd,
        "wo": Wo.astype(BF),
        "eyeb": eye, "gmat": g, "mks": masks,
    }
    in_maps = [
        {"xt": np.ascontiguousarray(X[i].T), **shared} for i in range(B)
    ]
    nc = _get_nc()
    r = run_bass_kernel_spmd(nc, in_maps, list(range(B)))
    return np.stack([r.results[i]["out"] for i in range(B)]).astype(np.float32)


# revision 15
# speedup vs baseline: 1.0586x; 1.0586x over previous
"""AttentionLSEncoder (landmark + sliding-window attention) on 8 TRN2
NeuronCores, data-parallel over the batch (b=8 -> 1 element per core).

kernel(**inputs) takes the FULL unsharded inputs and returns the FULL
[8, 2000, 512] float32 output.
"""
import numpy as np
import ml_dtypes

import concourse.bass as bass
import concourse.tile as tile
from concourse import mybir
from concourse.bass_utils import run_bass_kernel_spmd

# ----------------------------------------------------------------------------
# Workaround: this container's walrus build accepts only ONE sync-wait per
# instruction. Split multi-wait instructions into single-wait NoOp chains.
# ----------------------------------------------------------------------------
from concourse.vector_clock import ScopedClock

_orig_add = tile.TileContext._add_instruction


def _split_waits_engine(self, inst):
    si = getattr(inst, "sync_info", None)
    if si is None or not si.on_wait or len(si.on_wait) <= 1:
        return
    eng = inst.engine
    if eng is None or eng == mybir.EngineType.Unassigned:
        return
    waits = list(si.on_wait)
    for i, w in enumerate(waits[:-1]):
        nop = mybir.InstNoOp(
            name=f"{inst.name}-wsplit{i}",
            sync_info=mybir.SyncInfo(on_wait=[w], on_update=[]),
            bass_nofuse=True,
            engine=eng,
        )
        _orig_add(self, nop)
    inst.sync_info = mybir.SyncInfo(
        on_wait=[waits[-1]], on_update=list(si.on_update or [])
    )


def _add_instruction_split(self, inst):
    _split_waits_engine(self, inst)
    _orig_add(self, inst)


def _drain_and_barrier_split(self, tick_clock, wait_clock):
    nc = self.nc
    drain_inst = nc.sync.drain()
    wait_clock.add_sem_waits(
        drain_inst.ins, ScopedClock({None: tick_clock.global_clock})
    )
    si = drain_inst.ins.sync_info
    waits = list(si.on_wait) if si and si.on_wait else []
    if len(waits) > 1:
        drain_inst.ins.sync_info = mybir.SyncInfo(
            on_wait=waits[:1], on_update=list(si.on_update or [])
        )
        for w in waits[1:]:
            extra = nc.sync.drain()
            extra.ins.sync_info = mybir.SyncInfo(on_wait=[w], on_update=[])

    nc.all_engine_barrier()
    assert self.sems is not None
    popped = nc._tile_sem_poison_stack.pop()
    assert popped is self._sem_poison
    nc.clear_and_free_semaphores(list(self.sems.allocated().values()))
    nc.all_engine_barrier()


tile.TileContext._add_instruction = _add_instruction_split
tile.TileContext._drain_and_barrier = _drain_and_barrier_split

# ----------------------------------------------------------------------------
# Problem constants (hardcoded per the module definition)
# ----------------------------------------------------------------------------
B = 8
S = 2000
DM = 512
H = 4
DK = 512
HD = H * DK          # 2048
NL = 32
HL = H * NL          # 128
WS = 10
EXT = 5

NB = 20              # attention blocks
BT = 100             # tokens per block
BW = 110             # band keys per block
SW = NL + BW         # score width = 142
ND = DM // 128       # 4 contraction chunks
NF = HD // 128       # 16 feature chunks
NG = 4               # token groups of 500
GT = S // NG         # 500
SCALE = 1.0 / float(np.sqrt(DK))

# token tiles of 128 (last is 80)
TT = [(i * 128, min(128, S - i * 128)) for i in range((S + 127) // 128)]
NT = len(TT)  # 16

F32 = mybir.dt.float32
F32R = mybir.dt.float32r
BF16 = mybir.dt.bfloat16
BF = ml_dtypes.bfloat16


def _band_start(i):
    return min(max(BT * i - EXT, 0), S - BW)


def _host_consts():
    eye = np.eye(128, dtype=BF)
    g = np.zeros((WS, BT), dtype=BF)
    for w in range(BT):
        g[w // WS, w] = 1.0
    masks = np.full((WS, NB, SW), -30.0, dtype=np.float32)
    masks[:, :, :NL] = 0.0
    for i in range(NB):
        b0 = _band_start(i)
        for gw in range(WS):
            gg = WS * i + gw
            lo, hi = WS * gg - EXT, WS * gg + EXT + WS
            for j in range(BW):
                k = b0 + j
                if lo <= k < hi:
                    masks[gw, i, NL + j] = 0.0
    return eye, g, masks.astype(BF)


def build_nc(debug=False):
    nc = bass.Bass("TRN2", target_bir_lowering=False, debug=False)

    xt = nc.dram_tensor("xt", [DM, S], F32R, kind="ExternalInput")
    wq = nc.dram_tensor("wq", [DM, HD], F32R, kind="ExternalInput")
    wk = nc.dram_tensor("wk", [DM, HD], F32R, kind="ExternalInput")
    wv = nc.dram_tensor("wv", [DM, HD], F32R, kind="ExternalInput")
    wd = nc.dram_tensor("wd", [DM, HL], F32R, kind="ExternalInput")
    wo = nc.dram_tensor("wo", [HD, DM], BF16, kind="ExternalInput")
    eyeb = nc.dram_tensor("eyeb", [128, 128], BF16, kind="ExternalInput")
    gmat = nc.dram_tensor("gmat", [WS, BT], BF16, kind="ExternalInput")
    mks = nc.dram_tensor("mks", [WS, NB, SW], BF16, kind="ExternalInput")
    out = nc.dram_tensor("out", [S, DM], F32, kind="ExternalOutput")

    k_bf = nc.dram_tensor("k_bf", [S, HD], BF16, kind="ExternalOutput" if debug else "Internal")
    v_bf = nc.dram_tensor("v_bf", [S, HD], BF16, kind="ExternalOutput" if debug else "Internal")
    if debug:
        dbg_qt = nc.dram_tensor("dbg_qt", [128, NF, S], BF16, kind="ExternalOutput")
        dbg_hs = nc.dram_tensor("dbg_hs", [128, S], BF16, kind="ExternalOutput")
        dbg_rlm = nc.dram_tensor("dbg_rlm", [128, 1], F32, kind="ExternalOutput")
        dbg_kc = nc.dram_tensor("dbg_kc", [32, HD], BF16, kind="ExternalOutput")
        dbg_vc = nc.dram_tensor("dbg_vc", [32, HD], BF16, kind="ExternalOutput")
        dbg_kct = nc.dram_tensor("dbg_kct", [128, NF, NL], BF16, kind="ExternalOutput")
        dbg_kt = nc.dram_tensor("dbg_kt", [128, NF, S], BF16, kind="ExternalOutput")
        dbg_sc = nc.dram_tensor("dbg_sc", [BT, SW], F32, kind="ExternalOutput")
        dbg_e = nc.dram_tensor("dbg_e", [BT, SW], BF16, kind="ExternalOutput")
        dbg_et = nc.dram_tensor("dbg_et", [BW, 2, BT], BF16, kind="ExternalOutput")
        dbg_ct = nc.dram_tensor("dbg_ct", [128, NF, BT], BF16, kind="ExternalOutput")

    xt_r = xt.ap().rearrange("(c p) t -> p c t", p=128)

    with tile.TileContext(nc) as tc:
        with (
            tc.tile_pool(name="resid", bufs=1) as rp,
            tc.tile_pool(name="rpsum", bufs=1, space="PSUM") as _unused_rps,
        ):
            eye_sb = rp.tile([128, 128], BF16)
            nc.sync.dma_start(eye_sb[:], eyeb.ap())
            g_sb = rp.tile([WS, BT], BF16)
            nc.sync.dma_start(g_sb[:], gmat.ap())
            eps = rp.tile([128, 1], F32)
            nc.vector.memset(eps[:], 1e-5)

            qt = rp.tile([128, NF, S], BF16)
            kt = rp.tile([128, NF, S], BF16)
            hs_exp = rp.tile([128, S], BF16)
            recip_lm = rp.tile([128, 1], F32)
            kct = rp.tile([128, NF, NL], BF16)
            vc = rp.tile([32, HD], BF16)
            kc_tok = rp.tile([32, HD], BF16)

            # ================= S1a: K/V projections + LN, hs logits ======
            with (
                tc.tile_pool(name="s1a", bufs=1) as p1,
                tc.tile_pool(name="s1aw", bufs=2) as p1w,
                tc.tile_pool(name="ps1", bufs=2, space="PSUM") as ps1,
            ):
                wd_sb = p1.tile([128, ND, HL], F32R)
                nc.sync.dma_start(
                    wd_sb[:], wd.ap().rearrange("(c p) f -> p c f", p=128)
                )
                den4 = p1.tile([128, NG], F32)

                for tens_i, (wdram, odram) in enumerate(((wk, k_bf), (wv, v_bf))):
                    w_sb = p1.tile([128, ND, HD], F32R, tag="wtens")
                    nc.sync.dma_start(
                        w_sb[:], wdram.ap().rearrange("(c p) f -> p c f", p=128)
                    )
                    for ti, (t0, tw) in enumerate(TT):
                        xt_t = p1w.tile([128, ND, 128], F32R, tag="xt_t")
                        nc.sync.dma_start(xt_t[:, :, :tw], xt_r[:, :, t0 : t0 + tw])
                        pra = ps1.tile([128, 2, 512], F32, tag="projA")
                        prb = ps1.tile([128, 2, 512], F32, tag="projB")
                        halves = (pra, prb)
                        for g4 in range(ND):
                            pr = halves[g4 // 2]
                            for c in range(ND):
                                nc.tensor.matmul(
                                    pr[:tw, g4 % 2, :],
                                    xt_t[:, c, :tw],
                                    w_sb[:, c, 512 * g4 : 512 * (g4 + 1)],
                                    start=(c == 0),
                                    stop=(c == ND - 1),
                                )
                        st = p1w.tile([128, ND, 6], F32, tag="st")
                        for g4 in range(ND):
                            nc.vector.bn_stats(
                                st[:tw, g4, :], halves[g4 // 2][:tw, g4 % 2, :]
                            )
                        mv = p1w.tile([128, 2], F32, tag="mv")
                        nc.vector.bn_aggr(mv[:tw], st[:tw])
                        sd = p1w.tile([128, 1], F32, tag="sd")
                        nc.scalar.activation(
                            sd[:tw], mv[:tw, 1:2],
                            mybir.ActivationFunctionType.Sqrt, bias=eps[:tw],
                        )
                        rs = p1w.tile([128, 1], F32, tag="rs")
                        nc.vector.reciprocal(rs[:tw], sd[:tw])
                        nb = p1w.tile([128, 1], F32, tag="nb")
                        nc.vector.tensor_scalar(
                            nb[:tw], mv[:tw, 0:1], rs[:tw], -1.0,
                            mybir.AluOpType.mult, mybir.AluOpType.mult,
                        )
                        stg = p1w.tile([128, HD], BF16, tag="stg")
                        for half in range(2):
                            # (x - m) * r == x * r + (-m * r), on ACT
                            nc.scalar.activation(
                                stg[:tw, 1024 * half : 1024 * (half + 1)],
                                halves[half][:tw],
                                mybir.ActivationFunctionType.Identity,
                                bias=nb[:tw], scale=rs[:tw],
                            )
                        nc.sync.dma_start(odram[t0 : t0 + tw, :], stg[:tw])

                # landmark logits -> exp (feature-major [HL, S])
                for g4 in range(NG):
                    xt_g = p1w.tile([128, ND, GT], F32R, tag="xt_g")
                    nc.sync.dma_start(xt_g[:], xt_r[:, :, GT * g4 : GT * (g4 + 1)])
                    hp = ps1.tile([128, GT], F32, tag="projA")
                    for c in range(ND):
                        nc.tensor.matmul(
                            hp[:], wd_sb[:, c, :], xt_g[:, c, :],
                            start=(c == 0), stop=(c == ND - 1),
                        )
                    nc.scalar.activation(
                        hs_exp[:, GT * g4 : GT * (g4 + 1)], hp[:],
                        mybir.ActivationFunctionType.Exp,
                        accum_out=den4[:, g4 : g4 + 1],
                    )
                dsum = p1.tile([128, 1], F32)
                nc.vector.reduce_sum(dsum[:], den4[:], axis=mybir.AxisListType.X)
                nc.vector.reciprocal(recip_lm[:], dsum[:])

            # ================= S1b: Q^T projection (feature-major) =======
            with (
                tc.tile_pool(name="s1b", bufs=1) as p2,
                tc.tile_pool(name="s1bw", bufs=2) as p2w,
                tc.tile_pool(name="ps2", bufs=4, space="PSUM") as ps2,
            ):
                wq_sb = p2.tile([128, ND, HD], F32R)
                nc.sync.dma_start(
                    wq_sb[:], wq.ap().rearrange("(c p) f -> p c f", p=128)
                )
                for g4 in range(NG):
                    xt_g = p2w.tile([128, ND, GT], F32R, tag="xt_g")
                    nc.sync.dma_start(xt_g[:], xt_r[:, :, GT * g4 : GT * (g4 + 1)])
                    for j in range(NF):
                        qp = ps2.tile([128, GT], F32, tag="q")
                        for c in range(ND):
                            nc.tensor.matmul(
                                qp[:], wq_sb[:, c, 128 * j : 128 * (j + 1)],
                                xt_g[:, c, :],
                                start=(c == 0), stop=(c == ND - 1),
                            )
                        nc.scalar.activation(
                            qt[:, j, GT * g4 : GT * (g4 + 1)], qp[:],
                            mybir.ActivationFunctionType.Copy, scale=SCALE,
                        )

            # ================= S2: K^T, hs transpose, K_c/V_c ============
            with (
                tc.tile_pool(name="s2", bufs=1) as p3,
                tc.tile_pool(name="s2w", bufs=3) as p3w,
                tc.tile_pool(name="ps3", bufs=1, space="PSUM") as ps3,
                tc.tile_pool(name="ps3t", bufs=2, space="PSUM") as ps3t,
            ):
                # hs_exp [128, S] -> hs_tok [s, 128] per 128-token chunk
                hs_tok = p3.tile([128, NT, HL], BF16)
                for ti, (t0, tw) in enumerate(TT):
                    tp = ps3t.tile([128, 128], BF16, tag="tr")
                    nc.tensor.transpose(
                        tp[:tw, :], hs_exp[:, t0 : t0 + tw], eye_sb[:]
                    )
                    nc.scalar.copy(hs_tok[:tw, ti, :], tp[:tw, :])

                # K_c then V_c
                for tens_i, (src, dst) in enumerate(
                    ((k_bf, kc_tok), (v_bf, vc))
                ):
                    cv = ps3.tile([32, HD], F32, tag="cvec")
                    for ti, (t0, tw) in enumerate(TT):
                        ch = p3w.tile([128, HD], BF16, tag="chunk")
                        nc.sync.dma_start(ch[:tw], src[t0 : t0 + tw, :])
                        for h in range(H):
                            nc.tensor.matmul(
                                cv[:, 512 * h : 512 * (h + 1)],
                                hs_tok[:tw, ti, 32 * h : 32 * (h + 1)],
                                ch[:tw, 512 * h : 512 * (h + 1)],
                                start=(ti == 0), stop=(ti == NT - 1),
                            )
                        if tens_i == 0:
                            # K^T via on-chip PE transposes of each chunk
                            for j in range(NF):
                                tp = ps3t.tile([128, 128], BF16, tag="tr")
                                nc.tensor.transpose(
                                    tp[:, :tw],
                                    ch[:tw, 128 * j : 128 * (j + 1)],
                                    eye_sb[:tw, :tw],
                                )
                                nc.vector.tensor_copy(
                                    kt[:, j, t0 : t0 + tw], tp[:, :tw]
                                )
                    # scale by landmark softmax denominators, then LN
                    for h in range(H):
                        nc.vector.tensor_scalar(
                            cv[:, 512 * h : 512 * (h + 1)],
                            cv[:, 512 * h : 512 * (h + 1)],
                            recip_lm[32 * h : 32 * (h + 1), 0:1], None,
                            mybir.AluOpType.mult,
                        )
                    st2 = p3w.tile([32, ND, 6], F32, tag="st2")
                    for g4 in range(ND):
                        nc.vector.bn_stats(st2[:, g4, :], cv[:, 512 * g4 : 512 * (g4 + 1)])
                    mv2 = p3w.tile([32, 2], F32, tag="mv2")
                    nc.vector.bn_aggr(mv2[:], st2[:])
                    sd2 = p3w.tile([32, 1], F32, tag="sd2")
                    nc.scalar.activation(
                        sd2[:], mv2[:, 1:2],
                        mybir.ActivationFunctionType.Sqrt, bias=eps[:32],
                    )
                    rs2 = p3w.tile([32, 1], F32, tag="rs2")
                    nc.vector.reciprocal(rs2[:], sd2[:])
                    nc.vector.tensor_scalar(
                        dst[:], cv[:], mv2[:, 0:1], rs2[:],
                        mybir.AluOpType.subtract, mybir.AluOpType.mult,
                    )

                # K_c^T [f, l] for the score matmuls
                for j in range(NF):
                    tp = ps3t.tile([128, NL], BF16, tag="trc")
                    nc.tensor.transpose(
                        tp[:], kc_tok[:, 128 * j : 128 * (j + 1)], eye_sb[:32, :32]
                    )
                    nc.scalar.copy(kct[:, j, :], tp[:])
                if debug:
                    nc.sync.dma_start(dbg_qt.ap(), qt[:])
                    nc.sync.dma_start(dbg_hs.ap(), hs_exp[:])
                    nc.sync.dma_start(dbg_rlm.ap(), recip_lm[:])
                    nc.sync.dma_start(dbg_kc.ap(), kc_tok[:])
                    nc.sync.dma_start(dbg_vc.ap(), vc[:])
                    nc.sync.dma_start(dbg_kct.ap(), kct[:])
                    nc.sync.dma_start(dbg_kt.ap(), kt[:])

            # ================= S4: blocked attention + Wo ================
            with (
                tc.tile_pool(name="s4", bufs=1) as p4,
                tc.tile_pool(name="s4w", bufs=2) as p4w,
                tc.tile_pool(name="ps4", bufs=2, space="PSUM") as ps4,
                tc.tile_pool(name="ps4b", bufs=1, space="PSUM") as ps4b,
                tc.tile_pool(name="ps4o", bufs=1, space="PSUM") as ps4o,
            ):
                wo_sb = p4.tile([128, NF, DM], BF16)
                nc.sync.dma_start(
                    wo_sb[:], wo.ap().rearrange("(c p) f -> p c f", p=128)
                )
                m_sb = p4.tile([WS, NB, SW], BF16)
                nc.sync.dma_start(m_sb[:], mks.ap())

                for i in range(NB):
                    t0 = BT * i
                    b0 = _band_start(i)
                    vband = p4w.tile([BW, HD], BF16, tag="vband")
                    nc.sync.dma_start(vband[:], v_bf[b0 : b0 + BW, :])
                    # 128-col chunk stride keeps each head's 4 chunks in
                    # exactly one PSUM bank (start=True clears a whole bank
                    # on this HW, so one start per bank, first toucher).
                    ct = ps4o.tile([128, NF, 128], F32, tag="ct")
                    for h in range(H):
                        sc = ps4.tile([BT, SW], F32, tag="sc")
                        # mask first: start=True writes every score column,
                        # then all QK matmuls accumulate.
                        nc.tensor.matmul(
                            sc[:], g_sb[:], m_sb[:, i, :],
                            start=True, stop=False, skip_group_check=True,
                        )
                        for c4 in range(ND):
                            c = 4 * h + c4
                            nc.tensor.matmul(
                                sc[:, :NL], qt[:, c, t0 : t0 + BT], kct[:, c, :],
                                start=False, stop=False,
                                skip_group_check=True,
                            )
                            nc.tensor.matmul(
                                sc[:, NL:], qt[:, c, t0 : t0 + BT],
                                kt[:, c, b0 : b0 + BW],
                                start=False, stop=(c4 == ND - 1),
                                skip_group_check=True,
                            )
                        e_sb = p4w.tile([BT, SW], BF16, tag="e")
                        den = p4w.tile([BT, 1], F32, tag="den")
                        nc.scalar.activation(
                            e_sb[:], sc[:], mybir.ActivationFunctionType.Exp,
                            accum_out=den[:],
                        )
                        rec = p4w.tile([BT, 1], F32, tag="rec")
                        nc.vector.reciprocal(rec[:], den[:])
                        d_sb = p4w.tile([BT, BT], BF16, tag="d")
                        nc.vector.tensor_scalar(
                            d_sb[:], eye_sb[:BT, :BT], rec[:], None,
                            mybir.AluOpType.mult,
                        )
                        etp = ps4b.tile([BW, 2, BT], F32, tag="et")
                        # band transpose first: its start=True clears the
                        # bank; the landmark transpose then overwrites its
                        # own (cleared) region with start=False.
                        nc.tensor.matmul(
                            etp[:, 1, :], e_sb[:, NL:], d_sb[:],
                            start=True, stop=False, skip_group_check=True,
                        )
                        nc.tensor.matmul(
                            etp[:NL, 0, :], e_sb[:, :NL], d_sb[:],
                            start=False, stop=True, skip_group_check=True,
                        )
                        et_sb = p4w.tile([BW, 2, BT], BF16, tag="ets")
                        nc.scalar.copy(et_sb[:], etp[:])
                        if debug and i == 0 and h == 0:
                            scd = p4w.tile([BT, SW], F32, tag="scdump")
                            nc.scalar.copy(scd[:], sc[:])
                            nc.sync.dma_start(dbg_sc.ap(), scd[:])
                            nc.sync.dma_start(dbg_e.ap(), e_sb[:])
                            nc.sync.dma_start(dbg_et.ap(), et_sb[:])
                        for c4 in range(ND):
                            c = 4 * h + c4
                            d0 = 512 * h + 128 * c4
                            nc.tensor.matmul(
                                ct[:, c, :BT], vc[:, d0 : d0 + 128],
                                et_sb[:NL, 0, :],
                                start=(c4 == 0), stop=False,
                                skip_group_check=True,
                            )
                            nc.tensor.matmul(
                                ct[:, c, :BT], vband[:, d0 : d0 + 128],
                                et_sb[:, 1, :],
                                start=False, stop=(c4 == ND - 1),
                                skip_group_check=True,
                            )
                    ct_sb = p4w.tile([128, NF, BT], BF16, tag="cts")
                    nc.scalar.copy(ct_sb[:], ct[:, :, :BT])
                    if debug and i == 0:
                        nc.sync.dma_start(dbg_ct.ap(), ct_sb[:])
                    op = ps4o.tile([BT, DM], F32, tag="wo")
                    for c in range(NF):
                        nc.tensor.matmul(
                            op[:], ct_sb[:, c, :], wo_sb[:, c, :],
                            start=(c == 0), stop=(c == NF - 1),
                        )
                    o_sb = p4w.tile([BT, DM], F32, tag="osb")
                    nc.scalar.copy(o_sb[:], op[:])
                    nc.sync.dma_start(out[t0 : t0 + BT, :], o_sb[:])

    return nc


_NC_CACHE = {}


def _get_nc():
    if "nc" not in _NC_CACHE:
        _NC_CACHE["nc"] = build_nc()
    return _NC_CACHE["nc"]


def kernel(**inputs):
    X = np.asarray(inputs["X"], dtype=np.float32)
    Wq = np.asarray(inputs["Wq"], dtype=np.float32)
    Wk = np.asarray(inputs["Wk"], dtype=np.float32)
    Wv = np.asarray(inputs["Wv"], dtype=np.float32)
    Wd = np.asarray(inputs["Wd"], dtype=np.float32)
    Wo = np.asarray(inputs["Wo"], dtype=np.float32)

    eye, g, masks = _host_consts()
    shared = {
        "wq": Wq, "wk": Wk, "wv": Wv, "wd": Wd,
        "wo": Wo.astype(BF),
        "eyeb": eye, "gmat": g, "mks": masks,
    }
    in_maps = [
        {"xt": np.ascontiguousarray(X[i].T), **shared} for i in range(B)
    ]
    nc = _get_nc()
    r = run_bass_kernel_spmd(nc, in_maps, list(range(B)))
    return np.stack([r.results[i]["out"] for i in range(B)]).astype(np.float32)


# revision 19
# speedup vs baseline: 1.1052x; 1.0441x over previous
"""AttentionLSEncoder (landmark + sliding-window attention) on 8 TRN2
NeuronCores, data-parallel over the batch (b=8 -> 1 element per core).

kernel(**inputs) takes the FULL unsharded inputs and returns the FULL
[8, 2000, 512] float32 output.
"""
import numpy as np
import ml_dtypes

import concourse.bass as bass
import concourse.tile as tile
from concourse import mybir
from concourse.bass_utils import run_bass_kernel_spmd

# ----------------------------------------------------------------------------
# Workaround: this container's walrus build accepts only ONE sync-wait per
# instruction. Split multi-wait instructions into single-wait NoOp chains.
# ----------------------------------------------------------------------------
from concourse.vector_clock import ScopedClock

_orig_add = tile.TileContext._add_instruction


def _split_waits_engine(self, inst):
    si = getattr(inst, "sync_info", None)
    if si is None or not si.on_wait or len(si.on_wait) <= 1:
        return
    eng = inst.engine
    if eng is None or eng == mybir.EngineType.Unassigned:
        return
    waits = list(si.on_wait)
    for i, w in enumerate(waits[:-1]):
        nop = mybir.InstNoOp(
            name=f"{inst.name}-wsplit{i}",
            sync_info=mybir.SyncInfo(on_wait=[w], on_update=[]),
            bass_nofuse=True,
            engine=eng,
        )
        _orig_add(self, nop)
    inst.sync_info = mybir.SyncInfo(
        on_wait=[waits[-1]], on_update=list(si.on_update or [])
    )


def _add_instruction_split(self, inst):
    _split_waits_engine(self, inst)
    _orig_add(self, inst)


def _drain_and_barrier_split(self, tick_clock, wait_clock):
    nc = self.nc
    drain_inst = nc.sync.drain()
    wait_clock.add_sem_waits(
        drain_inst.ins, ScopedClock({None: tick_clock.global_clock})
    )
    si = drain_inst.ins.sync_info
    waits = list(si.on_wait) if si and si.on_wait else []
    if len(waits) > 1:
        drain_inst.ins.sync_info = mybir.SyncInfo(
            on_wait=waits[:1], on_update=list(si.on_update or [])
        )
        for w in waits[1:]:
            extra = nc.sync.drain()
            extra.ins.sync_info = mybir.SyncInfo(on_wait=[w], on_update=[])

    nc.all_engine_barrier()
    assert self.sems is not None
    popped = nc._tile_sem_poison_stack.pop()
    assert popped is self._sem_poison
    nc.clear_and_free_semaphores(list(self.sems.allocated().values()))
    nc.all_engine_barrier()


tile.TileContext._add_instruction = _add_instruction_split
tile.TileContext._drain_and_barrier = _drain_and_barrier_split

# ----------------------------------------------------------------------------
# Problem constants (hardcoded per the module definition)
# ----------------------------------------------------------------------------
B = 8
S = 2000
DM = 512
H = 4
DK = 512
HD = H * DK          # 2048
NL = 32
HL = H * NL          # 128
WS = 10
EXT = 5

NB = 20              # attention blocks
BT = 100             # tokens per block
BW = 110             # band keys per block
SW = NL + BW         # score width = 142
ND = DM // 128       # 4 contraction chunks
NF = HD // 128       # 16 feature chunks
NG = 4               # token groups of 500
GT = S // NG         # 500
SCALE = 1.0 / float(np.sqrt(DK))

# token tiles of 128 (last is 80)
TT = [(i * 128, min(128, S - i * 128)) for i in range((S + 127) // 128)]
NT = len(TT)  # 16

F32 = mybir.dt.float32
F32R = mybir.dt.float32r
BF16 = mybir.dt.bfloat16
BF = ml_dtypes.bfloat16


def _band_start(i):
    return min(max(BT * i - EXT, 0), S - BW)


def _host_consts():
    eye = np.eye(128, dtype=BF)
    g = np.zeros((WS, BT), dtype=BF)
    for w in range(BT):
        g[w // WS, w] = 1.0
    masks = np.full((WS, NB, SW), -30.0, dtype=np.float32)
    masks[:, :, :NL] = 0.0
    for i in range(NB):
        b0 = _band_start(i)
        for gw in range(WS):
            gg = WS * i + gw
            lo, hi = WS * gg - EXT, WS * gg + EXT + WS
            for j in range(BW):
                k = b0 + j
                if lo <= k < hi:
                    masks[gw, i, NL + j] = 0.0
    return eye, g, masks.astype(BF)


def build_nc(debug=False):
    nc = bass.Bass("TRN2", target_bir_lowering=False, debug=False)

    xt = nc.dram_tensor("xt", [DM, S], F32R, kind="ExternalInput")
    wq = nc.dram_tensor("wq", [DM, HD], F32R, kind="ExternalInput")
    wk = nc.dram_tensor("wk", [DM, HD], F32R, kind="ExternalInput")
    wv = nc.dram_tensor("wv", [DM, HD], F32R, kind="ExternalInput")
    wd = nc.dram_tensor("wd", [DM, HL], F32R, kind="ExternalInput")
    wo = nc.dram_tensor("wo", [HD, DM], BF16, kind="ExternalInput")
    eyeb = nc.dram_tensor("eyeb", [128, 128], BF16, kind="ExternalInput")
    gmat = nc.dram_tensor("gmat", [WS, BT], BF16, kind="ExternalInput")
    mks = nc.dram_tensor("mks", [WS, NB, SW], BF16, kind="ExternalInput")
    out = nc.dram_tensor("out", [S, DM], F32, kind="ExternalOutput")

    k_bf = nc.dram_tensor("k_bf", [S, HD], BF16, kind="ExternalOutput" if debug else "Internal")
    v_bf = nc.dram_tensor("v_bf", [S, HD], BF16, kind="ExternalOutput" if debug else "Internal")
    if debug:
        dbg_qt = nc.dram_tensor("dbg_qt", [128, NF, S], BF16, kind="ExternalOutput")
        dbg_hs = nc.dram_tensor("dbg_hs", [128, S], BF16, kind="ExternalOutput")
        dbg_rlm = nc.dram_tensor("dbg_rlm", [128, 1], F32, kind="ExternalOutput")
        dbg_kc = nc.dram_tensor("dbg_kc", [32, HD], BF16, kind="ExternalOutput")
        dbg_vc = nc.dram_tensor("dbg_vc", [32, HD], BF16, kind="ExternalOutput")
        dbg_kct = nc.dram_tensor("dbg_kct", [128, NF, NL], BF16, kind="ExternalOutput")
        dbg_kt = nc.dram_tensor("dbg_kt", [128, NF, S], BF16, kind="ExternalOutput")
        dbg_sc = nc.dram_tensor("dbg_sc", [BT, SW], F32, kind="ExternalOutput")
        dbg_e = nc.dram_tensor("dbg_e", [BT, SW], BF16, kind="ExternalOutput")
        dbg_et = nc.dram_tensor("dbg_et", [BW, 2, BT], BF16, kind="ExternalOutput")
        dbg_ct = nc.dram_tensor("dbg_ct", [128, NF, BT], BF16, kind="ExternalOutput")

    xt_r = xt.ap().rearrange("(c p) t -> p c t", p=128)

    with tile.TileContext(nc) as tc:
        with (
            tc.tile_pool(name="resid", bufs=1) as rp,
            tc.tile_pool(name="rpsum", bufs=1, space="PSUM") as _unused_rps,
        ):
            eye_sb = rp.tile([128, 128], BF16)
            nc.sync.dma_start(eye_sb[:], eyeb.ap())
            g_sb = rp.tile([WS, BT], BF16)
            nc.sync.dma_start(g_sb[:], gmat.ap())
            eps = rp.tile([128, 1], F32)
            nc.vector.memset(eps[:], 1e-5)

            qt = rp.tile([128, NF, S], BF16)
            kt = rp.tile([128, NF, S], BF16)
            hs_exp = rp.tile([128, S], BF16)
            recip_lm = rp.tile([128, 1], F32)
            kct = rp.tile([128, NF, NL], BF16)
            hs_tok = rp.tile([128, NT, HL], BF16)
            vc = rp.tile([32, HD], BF16)
            kc_tok = rp.tile([32, HD], BF16)

            # ================= S1a: K/V projections + LN, hs logits ======
            with (
                tc.tile_pool(name="s1a", bufs=1) as p1,
                tc.tile_pool(name="s1aw", bufs=2) as p1w,
                tc.tile_pool(name="ps1", bufs=2, space="PSUM") as ps1,
            ):
                wd_sb = p1.tile([128, ND, HL], F32R)
                nc.sync.dma_start(
                    wd_sb[:], wd.ap().rearrange("(c p) f -> p c f", p=128)
                )
                den4 = p1.tile([128, NG], F32)

                for tens_i, (wdram, odram) in enumerate(((wk, k_bf), (wv, v_bf))):
                    w_sb = p1.tile([128, ND, HD], F32R, tag="wtens")
                    nc.sync.dma_start(
                        w_sb[:], wdram.ap().rearrange("(c p) f -> p c f", p=128)
                    )
                    for ti, (t0, tw) in enumerate(TT):
                        xt_t = p1w.tile([128, ND, 128], F32R, tag="xt_t")
                        nc.sync.dma_start(xt_t[:, :, :tw], xt_r[:, :, t0 : t0 + tw])
                        pra = ps1.tile([128, 2, 512], F32, tag="projA")
                        prb = ps1.tile([128, 2, 512], F32, tag="projB")
                        halves = (pra, prb)
                        # c outer: 4 consecutive matmuls share the xt_t
                        # stationary operand
                        for c in range(ND):
                            for g4 in range(ND):
                                nc.tensor.matmul(
                                    halves[g4 // 2][:tw, g4 % 2, :],
                                    xt_t[:, c, :tw],
                                    w_sb[:, c, 512 * g4 : 512 * (g4 + 1)],
                                    start=(c == 0),
                                    stop=(c == ND - 1),
                                )
                        st = p1w.tile([128, ND, 6], F32, tag="st")
                        for g4 in range(ND):
                            nc.vector.bn_stats(
                                st[:tw, g4, :], halves[g4 // 2][:tw, g4 % 2, :]
                            )
                        mv = p1w.tile([128, 2], F32, tag="mv")
                        nc.vector.bn_aggr(mv[:tw], st[:tw])
                        sd = p1w.tile([128, 1], F32, tag="sd")
                        nc.scalar.activation(
                            sd[:tw], mv[:tw, 1:2],
                            mybir.ActivationFunctionType.Sqrt, bias=eps[:tw],
                        )
                        rs = p1w.tile([128, 1], F32, tag="rs")
                        nc.vector.reciprocal(rs[:tw], sd[:tw])
                        nb = p1w.tile([128, 1], F32, tag="nb")
                        nc.vector.tensor_scalar(
                            nb[:tw], mv[:tw, 0:1], rs[:tw], -1.0,
                            mybir.AluOpType.mult, mybir.AluOpType.mult,
                        )
                        stg = p1w.tile([128, HD], BF16, tag="stg")
                        for half in range(2):
                            # (x - m) * r == x * r + (-m * r), on ACT
                            nc.scalar.activation(
                                stg[:tw, 1024 * half : 1024 * (half + 1)],
                                halves[half][:tw],
                                mybir.ActivationFunctionType.Identity,
                                bias=nb[:tw], scale=rs[:tw],
                            )
                        nc.sync.dma_start(odram[t0 : t0 + tw, :], stg[:tw])

                # landmark logits -> exp (feature-major [HL, S])
                for g4 in range(NG):
                    xt_g = p1w.tile([128, ND, GT], F32R, tag="xt_g")
                    nc.sync.dma_start(xt_g[:], xt_r[:, :, GT * g4 : GT * (g4 + 1)])
                    hp = ps1.tile([128, GT], F32, tag="projA")
                    for c in range(ND):
                        nc.tensor.matmul(
                            hp[:], wd_sb[:, c, :], xt_g[:, c, :],
                            start=(c == 0), stop=(c == ND - 1),
                        )
                    nc.scalar.activation(
                        hs_exp[:, GT * g4 : GT * (g4 + 1)], hp[:],
                        mybir.ActivationFunctionType.Exp,
                        accum_out=den4[:, g4 : g4 + 1],
                    )
                dsum = p1.tile([128, 1], F32)
                nc.vector.reduce_sum(dsum[:], den4[:], axis=mybir.AxisListType.X)
                nc.vector.reciprocal(recip_lm[:], dsum[:])

            # ================= S1b: Q^T projection (feature-major) =======
            with (
                tc.tile_pool(name="s1b", bufs=1) as p2,
                tc.tile_pool(name="s1bw", bufs=2) as p2w,
                tc.tile_pool(name="ps2", bufs=4, space="PSUM") as ps2,
            ):
                wq_sb = p2.tile([128, ND, HD], F32R)
                nc.sync.dma_start(
                    wq_sb[:], wq.ap().rearrange("(c p) f -> p c f", p=128)
                )
                for g4 in range(NG):
                    xt_g = p2w.tile([128, ND, GT], F32R, tag="xt_g")
                    nc.sync.dma_start(xt_g[:], xt_r[:, :, GT * g4 : GT * (g4 + 1)])
                    for j in range(NF):
                        qp = ps2.tile([128, GT], F32, tag="q")
                        for c in range(ND):
                            nc.tensor.matmul(
                                qp[:], wq_sb[:, c, 128 * j : 128 * (j + 1)],
                                xt_g[:, c, :],
                                start=(c == 0), stop=(c == ND - 1),
                            )
                        nc.scalar.activation(
                            qt[:, j, GT * g4 : GT * (g4 + 1)], qp[:],
                            mybir.ActivationFunctionType.Copy, scale=SCALE,
                        )
                # hs_exp [128, S] -> hs_tok [s, 128] per 128-token chunk
                for ti, (t0, tw) in enumerate(TT):
                    tp = ps2.tile([128, 128], BF16, tag="tr")
                    nc.tensor.transpose(
                        tp[:tw, :], hs_exp[:, t0 : t0 + tw], eye_sb[:]
                    )
                    nc.scalar.copy(hs_tok[:tw, ti, :], tp[:tw, :])
                # K^T via on-chip PE transposes of K chunks
                for ti, (t0, tw) in enumerate(TT):
                    ch = p2w.tile([128, HD], BF16, tag="kchunk")
                    nc.sync.dma_start(ch[:tw], k_bf[t0 : t0 + tw, :])
                    for j in range(NF):
                        tp = ps2.tile([128, 128], BF16, tag="tr")
                        nc.tensor.transpose(
                            tp[:, :tw],
                            ch[:tw, 128 * j : 128 * (j + 1)],
                            eye_sb[:tw, :tw],
                        )
                        nc.vector.tensor_copy(
                            kt[:, j, t0 : t0 + tw], tp[:, :tw]
                        )

            # ================= S2: K^T, hs transpose, K_c/V_c ============
            with (
                tc.tile_pool(name="s2w", bufs=3) as p3w,
                tc.tile_pool(name="ps3", bufs=2, space="PSUM") as ps3,
            ):
                # K_c and V_c (separate psum slots, chains overlap)
                for tens_i, (src, dst) in enumerate(
                    ((k_bf, kc_tok), (v_bf, vc))
                ):
                    cv = ps3.tile([32, HD], F32, tag="cvec")
                    for ti, (t0, tw) in enumerate(TT):
                        ch = p3w.tile([128, HD], BF16, tag="chunk")
                        nc.sync.dma_start(ch[:tw], src[t0 : t0 + tw, :])
                        for h in range(H):
                            nc.tensor.matmul(
                                cv[:, 512 * h : 512 * (h + 1)],
                                hs_tok[:tw, ti, 32 * h : 32 * (h + 1)],
                                ch[:tw, 512 * h : 512 * (h + 1)],
                                start=(ti == 0), stop=(ti == NT - 1),
                            )
                    # scale by landmark softmax denominators, then LN
                    for h in range(H):
                        nc.vector.tensor_scalar(
                            cv[:, 512 * h : 512 * (h + 1)],
                            cv[:, 512 * h : 512 * (h + 1)],
                            recip_lm[32 * h : 32 * (h + 1), 0:1], None,
                            mybir.AluOpType.mult,
                        )
                    st2 = p3w.tile([32, ND, 6], F32, tag="st2")
                    for g4 in range(ND):
                        nc.vector.bn_stats(st2[:, g4, :], cv[:, 512 * g4 : 512 * (g4 + 1)])
                    mv2 = p3w.tile([32, 2], F32, tag="mv2")
                    nc.vector.bn_aggr(mv2[:], st2[:])
                    sd2 = p3w.tile([32, 1], F32, tag="sd2")
                    nc.scalar.activation(
                        sd2[:], mv2[:, 1:2],
                        mybir.ActivationFunctionType.Sqrt, bias=eps[:32],
                    )
                    rs2 = p3w.tile([32, 1], F32, tag="rs2")
                    nc.vector.reciprocal(rs2[:], sd2[:])
                    nc.vector.tensor_scalar(
                        dst[:], cv[:], mv2[:, 0:1], rs2[:],
                        mybir.AluOpType.subtract, mybir.AluOpType.mult,
                    )

                if debug:
                    nc.sync.dma_start(dbg_qt.ap(), qt[:])
                    nc.sync.dma_start(dbg_hs.ap(), hs_exp[:])
                    nc.sync.dma_start(dbg_rlm.ap(), recip_lm[:])
                    nc.sync.dma_start(dbg_kc.ap(), kc_tok[:])
                    nc.sync.dma_start(dbg_vc.ap(), vc[:])
                    nc.sync.dma_start(dbg_kt.ap(), kt[:])

            # ================= S4: blocked attention + Wo ================
            with (
                tc.tile_pool(name="s4", bufs=1) as p4,
                tc.tile_pool(name="s4w", bufs=2) as p4w,
                tc.tile_pool(name="ps4", bufs=2, space="PSUM") as ps4,
                tc.tile_pool(name="ps4b", bufs=1, space="PSUM") as ps4b,
                tc.tile_pool(name="ps4o", bufs=1, space="PSUM") as ps4o,
            ):
                wo_sb = p4.tile([128, NF, DM], BF16)
                nc.sync.dma_start(
                    wo_sb[:], wo.ap().rearrange("(c p) f -> p c f", p=128)
                )
                m_sb = p4.tile([WS, NB, SW], BF16)
                nc.sync.dma_start(m_sb[:], mks.ap())

                # K_c^T [f, l] for the score matmuls
                for j in range(NF):
                    tp = ps4b.tile([128, NL], BF16, tag="et")
                    nc.tensor.transpose(
                        tp[:], kc_tok[:, 128 * j : 128 * (j + 1)], eye_sb[:32, :32]
                    )
                    nc.scalar.copy(kct[:, j, :], tp[:])
                if debug:
                    nc.sync.dma_start(dbg_kct.ap(), kct[:])

                for i in range(NB):
                    t0 = BT * i
                    b0 = _band_start(i)
                    vband = p4w.tile([BW, HD], BF16, tag="vband")
                    nc.sync.dma_start(vband[:], v_bf[b0 : b0 + BW, :])
                    # 128-col chunk stride keeps each head's 4 chunks in
                    # exactly one PSUM bank (start=True clears a whole bank
                    # on this HW, so one start per bank, first toucher).
                    ct = ps4o.tile([128, NF, 128], F32, tag="ct")
                    for h in range(H):
                        sc = ps4.tile([BT, SW], F32, tag="sc")
                        # mask first: start=True writes every score column,
                        # then all QK matmuls accumulate.
                        nc.tensor.matmul(
                            sc[:], g_sb[:], m_sb[:, i, :],
                            start=True, stop=False, skip_group_check=True,
                        )
                        for c4 in range(ND):
                            c = 4 * h + c4
                            nc.tensor.matmul(
                                sc[:, :NL], qt[:, c, t0 : t0 + BT], kct[:, c, :],
                                start=False, stop=False,
                                skip_group_check=True,
                            )
                            nc.tensor.matmul(
                                sc[:, NL:], qt[:, c, t0 : t0 + BT],
                                kt[:, c, b0 : b0 + BW],
                                start=False, stop=(c4 == ND - 1),
                                skip_group_check=True,
                            )
                        e_sb = p4w.tile([BT, SW], BF16, tag="e")
                        den = p4w.tile([BT, 1], F32, tag="den")
                        nc.scalar.activation(
                            e_sb[:], sc[:], mybir.ActivationFunctionType.Exp,
                            accum_out=den[:],
                        )
                        rec = p4w.tile([BT, 1], F32, tag="rec")
                        nc.vector.reciprocal(rec[:], den[:])
                        d_sb = p4w.tile([BT, BT], BF16, tag="d")
                        nc.vector.tensor_scalar(
                            d_sb[:], eye_sb[:BT, :BT], rec[:], None,
                            mybir.AluOpType.mult,
                        )
                        etp = ps4b.tile([BW, 2, BT], F32, tag="et")
                        # band transpose first: its start=True clears the
                        # bank; the landmark transpose then overwrites its
                        # own (cleared) region with start=False.
                        nc.tensor.matmul(
                            etp[:, 1, :], e_sb[:, NL:], d_sb[:],
                            start=True, stop=False, skip_group_check=True,
                        )
                        nc.tensor.matmul(
                            etp[:NL, 0, :], e_sb[:, :NL], d_sb[:],
                            start=False, stop=True, skip_group_check=True,
                        )
                        et_sb = p4w.tile([BW, 2, BT], BF16, tag="ets")
                        nc.scalar.copy(et_sb[:], etp[:])
                        if debug and i == 0 and h == 0:
                            scd = p4w.tile([BT, SW], F32, tag="scdump")
                            nc.scalar.copy(scd[:], sc[:])
                            nc.sync.dma_start(dbg_sc.ap(), scd[:])
                            nc.sync.dma_start(dbg_e.ap(), e_sb[:])
                            nc.sync.dma_start(dbg_et.ap(), et_sb[:])
                        for c4 in range(ND):
                            c = 4 * h + c4
                            d0 = 512 * h + 128 * c4
                            nc.tensor.matmul(
                                ct[:, c, :BT], vc[:, d0 : d0 + 128],
                                et_sb[:NL, 0, :],
                                start=(c4 == 0), stop=False,
                                skip_group_check=True,
                            )
                            nc.tensor.matmul(
                                ct[:, c, :BT], vband[:, d0 : d0 + 128],
                                et_sb[:, 1, :],
                                start=False, stop=(c4 == ND - 1),
                                skip_group_check=True,
                            )
                    ct_sb = p4w.tile([128, NF, BT], BF16, tag="cts")
                    nc.scalar.copy(ct_sb[:], ct[:, :, :BT])
                    if debug and i == 0:
                        nc.sync.dma_start(dbg_ct.ap(), ct_sb[:])
                    op = ps4o.tile([BT, DM], F32, tag="wo")
                    for c in range(NF):
                        nc.tensor.matmul(
                            op[:], ct_sb[:, c, :], wo_sb[:, c, :],
                            start=(c == 0), stop=(c == NF - 1),
                        )
                    o_sb = p4w.tile([BT, DM], F32, tag="osb")
                    nc.scalar.copy(o_sb[:], op[:])
                    nc.sync.dma_start(out[t0 : t0 + BT, :], o_sb[:])

    return nc


_NC_CACHE = {}


def _get_nc():
    if "nc" not in _NC_CACHE:
        _NC_CACHE["nc"] = build_nc()
    return _NC_CACHE["nc"]


def kernel(**inputs):
    X = np.asarray(inputs["X"], dtype=np.float32)
    Wq = np.asarray(inputs["Wq"], dtype=np.float32)
    Wk = np.asarray(inputs["Wk"], dtype=np.float32)
    Wv = np.asarray(inputs["Wv"], dtype=np.float32)
    Wd = np.asarray(inputs["Wd"], dtype=np.float32)
    Wo = np.asarray(inputs["Wo"], dtype=np.float32)

    eye, g, masks = _host_consts()
    shared = {
        "wq": Wq, "wk": Wk, "wv": Wv, "wd": Wd,
        "wo": Wo.astype(BF),
        "eyeb": eye, "gmat": g, "mks": masks,
    }
    in_maps = [
        {"xt": np.ascontiguousarray(X[i].T), **shared} for i in range(B)
    ]
    nc = _get_nc()
    r = run_bass_kernel_spmd(nc, in_maps, list(range(B)))
    return np.stack([r.results[i]["out"] for i in range(B)]).astype(np.float32)
